# revision 12
# baseline (speedup 1.0000x reference)
"""Trainium2 Bass kernel: CrossAttentionFusion (dense transformer block pair).

Math notes (vs the reference):
  - seq_len-1 cross attention: softmax over a single key == 1, so
    mha1(q_in, kv_in) == kv_in @ (Wo@Wv).T + (Wo@bv + bo).  q/k projections are
    dead code; the two projections fuse into ONE 768x768 matmul (host-fused).
  - Transposed layout: activations live as [feature, batch]; matmuls are
    lhsT(=W.T, stationary) x rhs(=x.T, moving) -> out = (x@W.T).T.
    LayerNorm reduces over features (= partitions) with a ones-vector matmul on
    the PE; per-sample stats are broadcast back over partitions with K=1 ones
    matmuls.
  - Data-parallel over batch: 16384 rows -> 8 cores x 2048.
  - bf16 matmul operands (2x PE throughput vs f32), f32 PSUM accumulation.
  - Software pipeline with 1-strip skew; LN broadcast+apply for strip s-1 is
    emitted mid-attention of strip s so the PE never waits on the LN row-stat
    chain (which would also re-throttle the PE clock via HAM).
"""

import numpy as np
import ml_dtypes

import concourse.bass as bass
from concourse import bacc, tile, mybir
from concourse.bass_utils import run_bass_kernel_spmd

BF16 = ml_dtypes.bfloat16
DT_BF = mybir.dt.bfloat16
DT_F32 = mybir.dt.float32
AF = mybir.ActivationFunctionType
ALU = mybir.AluOpType

B_FULL, E, H = 16384, 768, 8
F = 4 * E  # 3072
N_CORES = 8
BS = B_FULL // N_CORES  # 2048
EPS = 1e-5
P = 128
KE = E // P  # 6
KF = F // P  # 24


def build(bs=BS, strip=512, sub=512, use_gelu=True, num_devices=N_CORES,
          sim_safe=False):
    """Emit the per-core Bass program (SPMD: same program on every core)."""
    nstrip = bs // strip
    nsub = strip // sub
    assert nstrip * strip == bs and nsub * sub == strip

    nc = bacc.Bacc(
        "TRN2", target_bir_lowering=False, debug=False, num_devices=num_devices
    )

    # ---- DRAM I/O ----
    d_img = nc.dram_tensor("imageT", [E, bs], DT_BF, kind="ExternalInput")
    d_txt = nc.dram_tensor("textT", [E, bs], DT_BF, kind="ExternalInput")
    d_watt = {
        "it": nc.dram_tensor("watt_it", [E, E], DT_BF, kind="ExternalInput"),
        "ti": nc.dram_tensor("watt_ti", [E, E], DT_BF, kind="ExternalInput"),
    }
    d_wfp = nc.dram_tensor("wfp", [2 * E, E], DT_BF, kind="ExternalInput")
    d_w1 = {
        p: nc.dram_tensor(f"w1_{p}", [E, F], DT_BF, kind="ExternalInput")
        for p in ("fi", "ft")
    }
    d_w2 = {
        p: nc.dram_tensor(f"w2_{p}", [F, E], DT_BF, kind="ExternalInput")
        for p in ("fi", "ft")
    }
    bias_specs = {
        "batt_it": KE, "g_img": KE, "b_img": KE, "b1_fi": KF, "b2_fi": KE,
        "batt_ti": KE, "g_txt": KE, "b_txt": KE, "b1_ft": KF, "b2_ft": KE,
        "bfp": KE, "g_fp": KE, "b_fp_ln": KE,
    }
    d_bias = {
        n: nc.dram_tensor(n, [P, k], DT_F32, kind="ExternalInput")
        for n, k in bias_specs.items()
    }
    d_out = nc.dram_tensor("outT", [E, bs], DT_F32, kind="ExternalOutput")

    def dview(d):  # [E|2E, bs] dram -> [p, kt, n] view
        return d.ap().rearrange("(kt p) n -> p kt n", p=P)

    with tile.TileContext(nc) as tc:
        from contextlib import ExitStack

        with ExitStack() as ctx:
            const = ctx.enter_context(tc.tile_pool(name="const", bufs=1))
            pin = ctx.enter_context(tc.tile_pool(name="pin", bufs=2))
            pwork = ctx.enter_context(tc.tile_pool(name="pwork", bufs=2))
            ph = ctx.enter_context(tc.tile_pool(name="ph", bufs=1))
            prow = ctx.enter_context(tc.tile_pool(name="prow", bufs=1))
            pst = ctx.enter_context(tc.tile_pool(name="pst", bufs=3))
            pps = ctx.enter_context(
                tc.tile_pool(name="pps", bufs=2, space=bass.MemorySpace.PSUM)
            )
            pdram = ctx.enter_context(
                tc.tile_pool(name="pdram", bufs=1, space=bass.MemorySpace.DRAM)
            )

            # ---- constants needed for SP1 start (small, DMA'd first) ----
            ones_sb = const.tile([P, P], DT_BF)
            nc.vector.memset(ones_sb[:], 1.0)
            eps_sb = const.tile([1, 1], DT_F32)
            nc.vector.memset(eps_sb[:], EPS)
            watt_sb = {
                pfx: const.tile(
                    [P, KE, E], DT_BF, tag=f"watt_{pfx}", name=f"watt_{pfx}"
                )
                for pfx in ("it", "ti")
            }
            for k in range(KE):
                nc.sync.dma_start(
                    watt_sb["it"][:, k, :], dview(d_watt["it"])[:, k, :]
                )
            bias_sb = {}
            for n, k in bias_specs.items():
                t = const.tile([P, k], DT_F32, tag=f"bias_{n}")
                nc.sync.dma_start(t[:], d_bias[n].ap())
                bias_sb[n] = t
            wfp_sb = const.tile([P, 2 * KE, E], DT_BF)
            # (watt_ti / wfp DMAs are emitted at SP2/SP3 start, see below)

            # ---- internal DRAM trunk: per-strip tiles for fine-grained deps --
            d_img2 = [
                pdram.tile([P, KE, strip], DT_BF, tag=f"img2_{s}", name=f"img2_{s}")
                for s in range(nstrip)
            ]
            d_txt2 = [
                pdram.tile([P, KE, strip], DT_BF, tag=f"txt2_{s}", name=f"txt2_{s}")
                for s in range(nstrip)
            ]

            # ---------- helpers ----------
            def load_strip_ext(dsrc, sl, tag):
                t = pin.tile([P, KE, strip], DT_BF, tag=tag, name=f"in_{tag}")
                for k in range(KE):  # per-k DMAs spread across HWDGE queues
                    nc.sync.dma_start(t[:, k, :], dsrc[:, k, sl])
                return t

            def load_strip_trunk(dtile, tag):
                t = pin.tile([P, KE, strip], DT_BF, tag=tag, name=f"in_{tag}")
                for k in range(KE):
                    nc.sync.dma_start(t[:, k, :], dtile[:, k, :])
                return t

            def dense_att(rhs_t, resid_t, w_sb, b_sb, mid_hook=None):
                """r[m] = (x @ Wc.T).T[m] + b[m] + resid[m]  (bf16 out)."""
                r = pwork.tile([P, KE, strip], DT_BF, tag="r1", name="r1")
                for m in range(KE):
                    ps = pps.tile([P, strip], DT_F32, tag="mm", bufs=2, name="ps")
                    for k in range(KE):
                        nc.tensor.matmul(
                            ps[:],
                            w_sb[:, k, m * P:(m + 1) * P],
                            rhs_t[:, k, :],
                            start=(k == 0),
                            stop=(k == KE - 1),
                        )
                    nc.vector.scalar_tensor_tensor(
                        r[:, m, :], ps[:], b_sb[:, m:m + 1], resid_t[:, m, :],
                        ALU.add, ALU.add,
                    )
                    if m == 3 and mid_hook is not None:
                        mid_hook()
                return r

            def ln_presum(r):
                """DVE feature pre-sums of r and r^2 -> [P,strip] bf16 pair."""
                s = pwork.tile([P, strip], DT_BF, tag="s", name="s")
                nc.vector.tensor_tensor(s[:], r[:, 0, :], r[:, 1, :], ALU.add)
                for k in range(2, KE):
                    nc.vector.tensor_tensor(s[:], s[:], r[:, k, :], ALU.add)
                sq = pwork.tile([P, strip], DT_BF, tag="sq", name="sq")
                tmp = pwork.tile([P, strip], DT_BF, tag="sqtmp", name="sqtmp")
                nc.vector.tensor_tensor(sq[:], r[:, 0, :], r[:, 0, :], ALU.mult)
                for k in range(1, KE):
                    nc.vector.tensor_tensor(tmp[:], r[:, k, :], r[:, k, :], ALU.mult)
                    nc.vector.tensor_tensor(sq[:], sq[:], tmp[:], ALU.add)
                return s, sq

            def ln_redrows(ssq):
                """PE partition-reduce + row-stat chain -> (mean, rstd) rows."""
                s, sq = ssq
                red0 = pps.tile([1, strip], DT_F32, tag="red0", bufs=1, name="red0")
                red1 = pps.tile([1, strip], DT_F32, tag="red1", bufs=1, name="red1")
                nc.tensor.matmul(red0[:], ones_sb[:, 0:1], s[:], start=True, stop=True)
                nc.tensor.matmul(red1[:], ones_sb[:, 0:1], sq[:], start=True, stop=True)
                mean_bf = prow.tile([1, strip], DT_BF, tag="mean", name="mean")
                nc.scalar.activation(mean_bf[:], red0[:], AF.Copy, scale=1.0 / E)
                msq = prow.tile([1, strip], DT_F32, tag="msq", name="msq")
                nc.vector.tensor_tensor(msq[:], mean_bf[:], mean_bf[:], ALU.mult)
                var = prow.tile([1, strip], DT_F32, tag="var", name="var")
                nc.vector.scalar_tensor_tensor(
                    var[:], red1[:], 1.0 / E, msq[:], ALU.mult, ALU.subtract
                )
                rstd_bf = prow.tile([1, strip], DT_BF, tag="rstdbf", name="rstdbf")
                if sim_safe:
                    std = prow.tile([1, strip], DT_F32, tag="std", name="std")
                    nc.scalar.activation(std[:], var[:], AF.Sqrt, bias=eps_sb[0:1, 0:1])
                    rstd = prow.tile([1, strip], DT_F32, tag="rstd", name="rstd")
                    nc.vector.reciprocal(rstd[:], std[:])
                    nc.vector.tensor_copy(rstd_bf[:], rstd[:])
                else:
                    nc.scalar.activation(
                        rstd_bf[:], var[:], AF.Abs_reciprocal_sqrt,
                        bias=eps_sb[0:1, 0:1],
                    )
                return mean_bf, rstd_bf

            def ln_bcast_apply(r, rows, out_emit):
                """PE K=1 broadcast of stats over partitions + DVE/ACT apply."""
                mean_bf, rstd_bf = rows
                mb = pps.tile([P, strip], DT_F32, tag="hps", bufs=2, name="mb")
                nc.tensor.matmul(mb[:], ones_sb[0:1, :], mean_bf[:], start=True, stop=True)
                rb = pps.tile([P, strip], DT_F32, tag="ops", bufs=2, name="rb")
                nc.tensor.matmul(rb[:], ones_sb[0:1, :], rstd_bf[:], start=True, stop=True)
                for k in range(KE):
                    t = pwork.tile([P, strip], DT_BF, tag="lnt", name="lnt")
                    nc.vector.tensor_tensor(t[:], r[:, k, :], mb[:], ALU.subtract)
                    nc.vector.tensor_tensor(t[:], t[:], rb[:], ALU.mult)
                    out_emit(k, t)

            def ln_to_x(r, rows, g_sb, b_sb):
                x = [
                    pwork.tile([P, strip], DT_BF, tag=f"xk{k}", name=f"x{k}")
                    for k in range(KE)
                ]

                def emit(k, t):
                    nc.scalar.activation(
                        x[k][:], t[:], AF.Identity,
                        bias=b_sb[:, k:k + 1], scale=g_sb[:, k:k + 1],
                    )

                ln_bcast_apply(r, rows, emit)
                return x

            def ffn(x, w1, w2, b1_sb, b2_sb, dtile):
                """dtile[:, m, :] = x + (gelu(x@W1.T+b1))@W2.T + b2."""
                for si in range(nsub):
                    ssl = slice(si * sub, (si + 1) * sub)
                    h = ph.tile([P, KF, sub], DT_BF, tag="h", name="h")
                    for m in range(KF):
                        hps = pps.tile([P, sub], DT_F32, tag="hps", bufs=2, name="hps")
                        for k in range(KE):
                            nc.tensor.matmul(
                                hps[:], w1[k][:, m * P:(m + 1) * P], x[k][:, ssl],
                                start=(k == 0), stop=(k == KE - 1),
                            )
                        nc.scalar.activation(
                            h[:, m, :], hps[:],
                            AF.Gelu if use_gelu else AF.Identity,
                            bias=b1_sb[:, m:m + 1],
                        )
                    for m in range(KE):
                        ops = pps.tile([P, sub], DT_F32, tag="ops", bufs=2, name="ops")
                        for k in range(KF):
                            nc.tensor.matmul(
                                ops[:], w2[k][:, m * P:(m + 1) * P], h[:, k, :],
                                start=(k == 0), stop=(k == KF - 1),
                            )
                        st = pst.tile([P, sub], DT_BF, tag="stg", name="stg")
                        nc.vector.scalar_tensor_tensor(
                            st[:], ops[:], b2_sb[:, m:m + 1], x[m][:, ssl],
                            ALU.add, ALU.add,
                        )
                        nc.sync.dma_start(dtile[:, m, ssl], st[:])

            def superphase(pfx, rhs_src, res_view, watt, b_att, g_ln, b_ln,
                           w1d, w2d, b1, b2, dout):
                """rhs_src: callable s -> rhs strip tile; res_view: dram view."""
                with tc.tile_pool(name=f"wffn_{pfx}", bufs=1) as wp:
                    w1 = [wp.tile([P, F], DT_BF, tag=f"w1_{k}", name=f"w1{k}")
                          for k in range(KE)]
                    w2 = [wp.tile([P, E], DT_BF, tag=f"w2_{k}", name=f"w2{k}")
                          for k in range(KF)]

                    def load_w():
                        for k in range(KE):
                            nc.sync.dma_start(w1[k][:], w1d.ap()[k * P:(k + 1) * P, :])
                        for k in range(KF):
                            nc.sync.dma_start(w2[k][:], w2d.ap()[k * P:(k + 1) * P, :])

                    pend = None  # (r, rows, strip idx) awaiting bcast/apply+ffn
                    xcur = [None]
                    for s in range(nstrip):
                        sl = slice(s * strip, (s + 1) * strip)
                        rhs_t = rhs_src(s)
                        res_t = load_strip_ext(res_view, sl, "res_in")

                        hook = None
                        if pend is not None:
                            rp, rowsp, _ = pend

                            def hook(rp=rp, rowsp=rowsp):
                                xcur[0] = ln_to_x(rp, rowsp, g_ln, b_ln)

                        r = dense_att(rhs_t, res_t, watt, b_att, mid_hook=hook)
                        ssq = ln_presum(r)
                        if s == 0 and pfx == "fi":
                            load_w()  # after the first strip's work is queued
                            for k in range(KE):
                                nc.sync.dma_start(
                                    watt_sb["ti"][:, k, :],
                                    dview(d_watt["ti"])[:, k, :],
                                )
                        if pend is not None:
                            _, _, sp = pend
                            ffn(xcur[0], w1, w2, b1, b2, dout[sp])
                        if s == 0 and pfx != "fi":
                            load_w()
                            for k in range(2 * KE):
                                nc.sync.dma_start(
                                    wfp_sb[:, k, :], dview(d_wfp)[:, k, :]
                                )
                        rows = ln_redrows(ssq)
                        pend = (r, rows, s)
                    rp, rowsp, sp = pend
                    x = ln_to_x(rp, rowsp, g_ln, b_ln)
                    ffn(x, w1, w2, b1, b2, dout[sp])

            # ---- SP1: image branch (kv = text, residual = image) ----
            superphase(
                "fi",
                lambda s: load_strip_ext(
                    dview(d_txt), slice(s * strip, (s + 1) * strip), "rhs_in"
                ),
                dview(d_img), watt_sb["it"],
                bias_sb["batt_it"], bias_sb["g_img"], bias_sb["b_img"],
                d_w1["fi"], d_w2["fi"], bias_sb["b1_fi"], bias_sb["b2_fi"],
                d_img2,
            )
            # ---- SP2: text branch (kv = img2, residual = text) ----
            superphase(
                "ft",
                lambda s: load_strip_trunk(d_img2[s], "rhs_in"),
                dview(d_txt), watt_sb["ti"],
                bias_sb["batt_ti"], bias_sb["g_txt"], bias_sb["b_txt"],
                d_w1["ft"], d_w2["ft"], bias_sb["b1_ft"], bias_sb["b2_ft"],
                d_txt2,
            )

            # ---- SP3: fused projection + LN + gelu ----
            with tc.tile_pool(name="sp3", bufs=3) as p3:
                outv = dview(d_out)

                def fp_finish(rp3, slp, rowsp):
                    def emit_out(k, t):
                        o = p3.tile([P, strip], DT_F32, tag="of32", name="of32")
                        nc.scalar.activation(
                            o[:], t[:],
                            AF.Gelu if use_gelu else AF.Identity,
                            bias=bias_sb["b_fp_ln"][:, k:k + 1],
                            scale=bias_sb["g_fp"][:, k:k + 1],
                        )
                        nc.sync.dma_start(outv[:, k, slp], o[:])

                    ln_bcast_apply(rp3, rowsp, emit_out)

                # 1-strip skew with in-loop hooks: red(s-1) after m1,
                # finish(s-1) after m5 -- PE never waits on the LN chain.
                stages = []  # per strip dict: r3, sl, ssq, rows
                for s in range(nstrip):
                    sl = slice(s * strip, (s + 1) * strip)
                    a_in = load_strip_trunk(d_img2[s], "rhs_in")
                    b_in = load_strip_trunk(d_txt2[s], "res_in")
                    r3 = pwork.tile([P, KE, strip], DT_BF, tag="r1", name="r3")
                    for m in range(KE):
                        zps = pps.tile([P, strip], DT_F32, tag="mm", bufs=2, name="zps")
                        for k in range(2 * KE):
                            src = a_in if k < KE else b_in
                            nc.tensor.matmul(
                                zps[:], wfp_sb[:, k, m * P:(m + 1) * P],
                                src[:, k % KE, :],
                                start=(k == 0), stop=(k == 2 * KE - 1),
                            )
                        nc.scalar.activation(
                            r3[:, m, :], zps[:], AF.Identity,
                            bias=bias_sb["bfp"][:, m:m + 1],
                        )
                        if m == 1 and stages and "rows" not in stages[-1]:
                            stages[-1]["rows"] = ln_redrows(stages[-1]["ssq"])
                        if m == 5 and stages and not stages[-1].get("done"):
                            st1 = stages[-1]
                            fp_finish(st1["r3"], st1["sl"], st1["rows"])
                            st1["done"] = True
                    stages.append({"r3": r3, "sl": sl, "ssq": ln_presum(r3)})
                last = stages[-1]
                last["rows"] = ln_redrows(last["ssq"])
                fp_finish(last["r3"], last["sl"], last["rows"])

    nc.compile()
    return nc


# ---------------- host side ----------------

_BUILT = {}


def _get_nc(key):
    if key not in _BUILT:
        _BUILT[key] = build(*key)
    return _BUILT[key]


def _packv(v, ktiles):
    return np.ascontiguousarray(np.asarray(v, np.float32).reshape(ktiles, P).T)


def prep_inputs(inputs, bs=BS, n_cores=N_CORES):
    f32 = np.float32
    g = lambda n: np.asarray(inputs[n], f32)
    common = {}
    for pfx in ("it", "ti"):
        wc = g(f"{pfx}_Wo") @ g(f"{pfx}_Wv")
        bc = g(f"{pfx}_Wo") @ g(f"{pfx}_bv") + g(f"{pfx}_bo")
        common[f"watt_{pfx}"] = np.ascontiguousarray(wc.T).astype(BF16)
        common[f"batt_{pfx}"] = _packv(bc, KE)
    common["w1_fi"] = np.ascontiguousarray(g("fi_W1").T).astype(BF16)
    common["w2_fi"] = np.ascontiguousarray(g("fi_W2").T).astype(BF16)
    common["w1_ft"] = np.ascontiguousarray(g("ft_W1").T).astype(BF16)
    common["w2_ft"] = np.ascontiguousarray(g("ft_W2").T).astype(BF16)
    common["wfp"] = np.ascontiguousarray(g("fp_W").T).astype(BF16)
    common["b1_fi"] = _packv(g("fi_b1"), KF)
    common["b2_fi"] = _packv(g("fi_b2"), KE)
    common["b1_ft"] = _packv(g("ft_b1"), KF)
    common["b2_ft"] = _packv(g("ft_b2"), KE)
    common["bfp"] = _packv(g("fp_b"), KE)
    common["g_img"] = _packv(g("ln_img_g"), KE)
    common["b_img"] = _packv(g("ln_img_b"), KE)
    common["g_txt"] = _packv(g("ln_text_g"), KE)
    common["b_txt"] = _packv(g("ln_text_b"), KE)
    common["g_fp"] = _packv(g("fp_ln_g"), KE)
    common["b_fp_ln"] = _packv(g("fp_ln_b"), KE)

    imgT = g("image_embed").T.astype(BF16)
    txtT = g("text_embed").T.astype(BF16)
    in_maps = []
    for c in range(n_cores):
        sl = slice(c * bs, (c + 1) * bs)
        m = dict(common)
        m["imageT"] = np.ascontiguousarray(imgT[:, sl])
        m["textT"] = np.ascontiguousarray(txtT[:, sl])
        in_maps.append(m)
    return in_maps


CFG = (BS, 512, 512, True, N_CORES)


def kernel(**inputs):
    nc = _get_nc(CFG)
    in_maps = prep_inputs(inputs)
    res = run_bass_kernel_spmd(nc, in_maps, core_ids=list(range(N_CORES)))
    out = np.concatenate(
        [res.results[c]["outT"] for c in range(N_CORES)], axis=1
    )  # [E, B]
    return np.ascontiguousarray(out.T).astype(np.float32)
